# revision 1
# baseline (speedup 1.0000x reference)
"""Bass/Trainium2 kernel for nn_Bbox_loss (masked gather + smooth-L1 loss).

Sharding: 8 cores = 4 batches x 2 channel-halves. Core c handles batch
b = c//2 and global channels [3h, 3h+3) with h = c%2 (a contiguous slice
of pred[b]). The host re-lays the per-core pred slice channel-last
(a, d, hh, w, c) so the 3 channel values of one anchor are contiguous;
the device then needs only 3 indirect row-gather DMAs (one per FPN
level, 128 anchors x 3 contiguous f32 each), computes smooth-L1 against
diff, masks padded anchors (via smooth_l1(|e|*mask): smooth_l1(0) == 0),
and reduces to a partial (loss, mask_count). Host sums the 8 partials;
weight = mask_total / 2 (both halves of a batch count the same mask).

The w coordinate is pre-folded on the host into w3b = 3*w + base_l (the
channel-last layout scales flat offsets by 3; base_l is the level's flat
offset). The device clamp max(w3b, 0) keeps padded rows (-1 coords) at a
safe in-range index whose garbage value is masked out.
"""

import numpy as np

import concourse.bacc as bacc
import concourse.bass as bass
import concourse.mybir as mybir
import concourse.tile as tile
from concourse import bass_utils

B, M, A = 4, 128, 3
LEVEL_DIMS = (96, 48, 24)
N_CORES = 8
N_LVL = 3
C_HALF = 3  # channels per half

# per-level flat sizes of the per-core pred slice (9 rows of S^3 each)
_LVL_SIZES = tuple(9 * s**3 for s in LEVEL_DIMS)
_LVL_BASE = (0, _LVL_SIZES[0], _LVL_SIZES[0] + _LVL_SIZES[1])
NP_TOT = sum(_LVL_SIZES)
# 2-D view of the flat pred slice (DMA APs need >=2 dims; flat order kept,
# gather indices stay flat element indices because coef(axis=1) == 1)
PRED_COLS = 512
# one extra all-zero row: padded anchors gather rows of zeros from NP_TOT
PRED_ROWS = NP_TOT // PRED_COLS + 1
assert (PRED_ROWS - 1) * PRED_COLS == NP_TOT

# meta input columns (all int32; diff is f32 bit-cast)
# coords are level-major (l*4 + comp); comp = (a, d, h, w3b)
_C_COORD = 0    # 12 cols
_C_SVEC = 12    # 3 cols: per-level S (Horner multiplier, steps 1-2)
_C_SVEC3 = 15   # 3 cols: per-level 3*S (last Horner multiplier)
_C_DIFF = 18    # 9 cols: diff values (f32 bits), col = 3*l + c
META_COLS = 27

_F32 = mybir.dt.float32
_I32 = mybir.dt.int32

_BUILD_CACHE = {}


def _build():
    """Build + compile the (shared SPMD) Bass module once per process."""
    if "nc" in _BUILD_CACHE:
        return _BUILD_CACHE["nc"]

    nc = bacc.Bacc(
        "TRN2", target_bir_lowering=False, debug=False, num_devices=N_CORES
    )
    pred_h = nc.dram_tensor(
        "pred", [PRED_ROWS, PRED_COLS], _F32, kind="ExternalInput"
    )
    meta_h = nc.dram_tensor("meta", [M, META_COLS], _I32, kind="ExternalInput")
    out_h = nc.dram_tensor("out", [1, 2], _F32, kind="ExternalOutput")

    op = mybir.AluOpType
    with tile.TileContext(nc) as tc:
        with (
            tc.tile_pool(name="sb", bufs=1) as pool,
            tc.tile_pool(name="pp", bufs=1, space="PSUM") as psum_pool,
        ):
            ct = pool.tile([M, META_COLS], _I32)
            nc.sync.dma_start(out=ct[:], in_=meta_h.ap())

            sv = ct[:, _C_SVEC : _C_SVEC + 3]
            sv3 = ct[:, _C_SVEC3 : _C_SVEC3 + 3]
            dt = ct[:, _C_DIFF : _C_DIFF + 9].bitcast(_F32)
            coords = ct[:, _C_COORD : _C_COORD + 12].rearrange(
                "p (l c) -> p l c", c=4
            )

            ps = pool.tile([M, 2], _F32)

            # index chain first — everything downstream waits on the gathers.
            # Each coordinate's clamp max(.,0) fuses into its Horner step:
            # ridx = ((a'*S + d')*S + h')*(3S) + w3b'   (x' = max(x, 0))
            with tc.high_priority():
                lin = pool.tile([M, N_LVL], _I32)
                nc.vector.scalar_tensor_tensor(
                    out=lin[:], in0=coords[:, :, 0], scalar=0, in1=sv,
                    op0=op.max, op1=op.mult,
                )
                nc.vector.scalar_tensor_tensor(
                    out=lin[:], in0=coords[:, :, 1], scalar=0, in1=lin[:],
                    op0=op.max, op1=op.add,
                )
                nc.vector.tensor_tensor(
                    out=lin[:], in0=lin[:], in1=sv, op=op.mult
                )
                nc.vector.scalar_tensor_tensor(
                    out=lin[:], in0=coords[:, :, 2], scalar=0, in1=lin[:],
                    op0=op.max, op1=op.add,
                )
                nc.vector.tensor_tensor(
                    out=lin[:], in0=lin[:], in1=sv3, op=op.mult
                )
                nc.vector.scalar_tensor_tensor(
                    out=lin[:], in0=coords[:, :, 3], scalar=0, in1=lin[:],
                    op0=op.max, op1=op.add,
                )

            # 3 row-gathers: one per level, 128 rows x 3 contiguous f32
            gt = pool.tile([M, 9], _F32)
            for l in range(N_LVL):
                nc.gpsimd.indirect_dma_start(
                    out=gt[:, 3 * l : 3 * l + 3],
                    out_offset=None,
                    in_=pred_h.ap(),
                    in_offset=bass.IndirectOffsetOnAxis(
                        ap=lin[:, l : l + 1], axis=1
                    ),
                )

            # mask[p, l] = coord_a > -1 (pre-clamp), as f32 0/1
            # (scheduled into the DVE-idle window while the gathers run)
            mask = pool.tile([M, N_LVL], _F32)
            nc.vector.tensor_scalar(
                out=mask[:],
                in0=coords[:, :, 0],
                scalar1=-1,
                scalar2=None,
                op0=op.is_gt,
            )
            nc.vector.tensor_reduce(
                out=ps[:, 1:2],
                in_=mask[:],
                axis=mybir.AxisListType.X,
                op=op.add,
            )

            # smooth-L1; padded rows need no masking: they gather zeros
            # from the pred pad row and their diff entries are zeroed on
            # the host, so e = 0 and smooth_l1(0) = 0
            e = pool.tile([M, 9], _F32)
            nc.vector.tensor_sub(out=e[:], in0=gt[:], in1=dt)
            ae = pool.tile([M, 9], _F32)
            nc.vector.scalar_tensor_tensor(
                out=ae[:], in0=e[:], scalar=-1.0, in1=e[:],
                op0=op.mult, op1=op.max,
            )
            mt = pool.tile([M, 9], _F32)
            nc.vector.tensor_scalar(
                out=mt[:], in0=ae[:], scalar1=1.0, scalar2=None, op0=op.min
            )
            hq = pool.tile([M, 9], _F32)
            nc.vector.scalar_tensor_tensor(
                out=hq[:], in0=mt[:], scalar=0.5, in1=mt[:],
                op0=op.mult, op1=op.mult,
            )
            t1 = pool.tile([M, 9], _F32)
            nc.vector.scalar_tensor_tensor(
                out=t1[:], in0=mt[:], scalar=-1.0, in1=ae[:],
                op0=op.mult, op1=op.add,
            )
            v = pool.tile([M, 9], _F32)
            nc.vector.scalar_tensor_tensor(
                out=v[:], in0=t1[:], scalar=1.0, in1=hq[:],
                op0=op.mult, op1=op.add,
                accum_out=ps[:, 0:1],
            )

            # partition reduce via matmul with ones
            ones = pool.tile([M, 1], _F32)
            nc.vector.memset(ones[:], 1.0)
            acc = psum_pool.tile([1, 2], _F32)
            nc.tensor.matmul(
                out=acc[:], lhsT=ones[:], rhs=ps[:], start=True, stop=True
            )
            osb = pool.tile([1, 2], _F32)
            nc.vector.tensor_copy(out=osb[:], in_=acc[:])
            nc.sync.dma_start(out=out_h.ap(), in_=osb[:])

    nc.compile()
    _BUILD_CACHE["nc"] = nc
    return nc


def _shard(inputs):
    """Build the 8 per-core input maps from the full inputs."""
    preds = [np.ascontiguousarray(inputs[f"pred_l{l}"], dtype=np.float32)
             for l in range(N_LVL)]
    coords = [np.ascontiguousarray(inputs[f"coord_l{l}"], dtype=np.int32)
              for l in range(N_LVL)]
    diffs = [np.ascontiguousarray(inputs[f"diff_l{l}"], dtype=np.float32)
             for l in range(N_LVL)]

    in_maps = []
    for c in range(N_CORES):
        b, h = divmod(c, 2)
        # channel-last relayout: block (3c, 3a, S^3) -> (3a, S^3, 3c)
        blocks = []
        for l in range(N_LVL):
            s3 = LEVEL_DIMS[l] ** 3
            blk = preds[l][b, 9 * h : 9 * h + 9].reshape(C_HALF, A, s3)
            blocks.append(blk.transpose(1, 2, 0).reshape(-1))
        blocks.append(np.zeros(PRED_COLS, dtype=np.float32))
        pred_flat = np.concatenate(blocks).reshape(PRED_ROWS, PRED_COLS)

        meta = np.empty((M, META_COLS), dtype=np.int32)
        for l in range(N_LVL):
            meta[:, _C_COORD + 4 * l : _C_COORD + 4 * l + 4] = coords[l][b]
            # fold *3 + level base into the w coordinate (stays <0 for
            # padded rows only when 3*(-1)+base < 0, i.e. level 0 -> the
            # device clamp keeps every padded index in range)
            padded = coords[l][b][:, 0] < 0
            meta[:, _C_COORD + 4 * l + 3] = np.where(
                padded, NP_TOT, coords[l][b][:, 3] * 3 + _LVL_BASE[l]
            )
            meta[:, _C_SVEC + l] = LEVEL_DIMS[l]
            meta[:, _C_SVEC3 + l] = 3 * LEVEL_DIMS[l]
            meta[:, _C_DIFF + 3 * l : _C_DIFF + 3 * l + 3] = (
                diffs[l][b, :, 3 * h : 3 * h + 3]
                * (~padded)[:, None]
            ).view(np.int32)
        in_maps.append({"pred": pred_flat, "meta": meta})
    return in_maps


def run(inputs, trace=False, **kw):
    nc = _build()
    in_maps = _shard(inputs)
    res = bass_utils.run_bass_kernel_spmd(
        nc, in_maps, core_ids=list(range(N_CORES)), trace=trace, **kw
    )
    partials = np.stack([res.results[c]["out"][0] for c in range(N_CORES)])
    loss = np.float32(partials[:, 0].sum())
    weight = np.float32(partials[:, 1].sum() / 2.0)
    return (
        np.array([loss], dtype=np.float32),
        np.array([weight], dtype=np.float32),
    ), res


def kernel(**inputs):
    out, _ = run(inputs, trace=False)
    return out



# revision 7
# speedup vs baseline: 1.2833x; 1.2833x over previous
"""Bass/Trainium2 kernel for nn_Bbox_loss (masked gather + smooth-L1 loss).

Sharding: 8 cores = 4 batches x 2 anchor-halves. Core c handles batch
b = c//2 and anchors [64*mh, 64*mh+64) with mh = c%2, across all 3 FPN
levels and all 6 channels. The host re-lays each batch's pred
channel-last (a, voxel, 6ch) per level so one anchor-level's 6 channel
values are contiguous, and precomputes the flat gather index per
(anchor, level) — padded anchors point at an all-zero pad row past the
data.

A core's work is 64 anchors x 3 levels = 192 gather chunks of 6 f32.
The HW indirect DMA (SWDGE on Pool) generates ONE descriptor per out
partition row (out row bytes contiguous per index), so the 192 chunks
take 2 instructions: chunks 0-127 -> rows 0-127 cols [4:10) of the meta
tile, chunks 128-191 -> rows 0-63 cols [10:16). Both accumulate over
preloaded -diff (e = g - d via DMA compute op; the sign never matters
because smooth-L1 only uses |e|). Unused cells hold -diff = 0 and are
never gathered into, so they contribute e = 0.

Then 3 DVE ops over e [128, 12]:
  s0 = sum |e|                  (scalar_tensor_tensor mult/max, accum)
  r  = 0.5 * min(|e|, 1)        (tensor_scalar min+mult)
  s1 = sum (r - 1) * r          (scalar_tensor_tensor add/mult, accum)
per-partition loss = s0 + 2*s1 since smooth_l1 = |e| + 0.5m^2 - m with
m = min(|e|,1) = 2r. A free-axis reduce of the mask gives the valid
count. The [M, 3] partials (s0, s1, count) DMA out; the host sums the
128 partitions and 8 cores (the scalar all-reduce step). Chunks are
disjoint across cores, so no double counting.

No PE/Activation ops, no PSUM: only SP (direct DMAs), Pool (indirect
gathers) and DVE run, keeping the instruction stream and semaphore
traffic minimal.
"""

import numpy as np

import concourse.bacc as bacc
import concourse.bass as bass
import concourse.mybir as mybir
import concourse.tile as tile
from concourse import bass_utils

B, M, A = 4, 128, 3
LEVEL_DIMS = (96, 48, 24)
N_CORES = 8
N_LVL = 3
N_CH = 6

# per-level flat sizes of the per-batch channel-last pred (A*S^3 chunks
# of 6 f32)
_LVL_SIZES = tuple(N_CH * A * s**3 for s in LEVEL_DIMS)
_LVL_BASE = (0, _LVL_SIZES[0], _LVL_SIZES[0] + _LVL_SIZES[1])
NP_TOT = sum(_LVL_SIZES)
# 2-D view of the flat pred (DMA APs need >=2 dims; flat order kept,
# gather indices stay flat element indices because coef(axis=1) == 1)
PRED_COLS = 512
# one extra all-zero row: padded anchors gather 6 zeros from NP_TOT
PRED_ROWS = NP_TOT // PRED_COLS + 1
assert (PRED_ROWS - 1) * PRED_COLS == NP_TOT

# chunk k = 3*local_anchor + level, k in [0, 192)
N_CHUNK = 2 * M * N_LVL // N_CORES * 2  # 192 per core
assert N_CHUNK == 192

# meta tile columns (int32; mask/diff are f32 bit-cast)
_C_LIN0 = 0    # chunk p index (p = 0..127)
_C_LIN1 = 1    # chunk 128+p index (p = 0..63)
_C_MASK = 2    # 2 cols: mask of chunk p, chunk 128+p
_C_E0 = 4      # 6 cols: -diff of chunk p / gather1 dest
_C_E1 = 10     # 6 cols: -diff of chunk 128+p / gather2 dest
META_COLS = 16

_F32 = mybir.dt.float32
_I32 = mybir.dt.int32

_BUILD_CACHE = {}


def _build():
    """Build + compile the (shared SPMD) Bass module once per process."""
    if "nc" in _BUILD_CACHE:
        return _BUILD_CACHE["nc"]

    nc = bacc.Bacc(
        "TRN2", target_bir_lowering=False, debug=False, num_devices=N_CORES
    )
    pred_h = nc.dram_tensor(
        "pred", [PRED_ROWS, PRED_COLS], _F32, kind="ExternalInput"
    )
    meta_h = nc.dram_tensor("meta", [M, META_COLS], _I32, kind="ExternalInput")
    out_h = nc.dram_tensor("out", [M, 3], _F32, kind="ExternalOutput")

    op = mybir.AluOpType
    with tile.TileContext(nc) as tc:
        with tc.tile_pool(name="sb", bufs=1) as pool:
            ct = pool.tile([M, META_COLS], _I32)
            nc.sync.dma_start(out=ct[:], in_=meta_h.ap())

            ps = pool.tile([M, 3], _F32)

            # gathers: one descriptor per out partition row, 6 f32 each,
            # accumulated over the preloaded -diff => e = g - d
            with tc.high_priority():
                nc.gpsimd.indirect_dma_start(
                    out=ct[:, _C_E0 : _C_E0 + 6].bitcast(_F32),
                    out_offset=None,
                    in_=pred_h.ap(),
                    in_offset=bass.IndirectOffsetOnAxis(
                        ap=ct[:, _C_LIN0 : _C_LIN0 + 1], axis=1
                    ),
                    compute_op=op.add,
                )
                nc.gpsimd.indirect_dma_start(
                    out=ct[0:64, _C_E1 : _C_E1 + 6].bitcast(_F32),
                    out_offset=None,
                    in_=pred_h.ap(),
                    in_offset=bass.IndirectOffsetOnAxis(
                        ap=ct[0:64, _C_LIN1 : _C_LIN1 + 1], axis=1
                    ),
                    compute_op=op.add,
                )

            # mask count (no gather dep: runs on DVE while the gathers
            # are in flight)
            nc.vector.tensor_reduce(
                out=ps[:, 2:3],
                in_=ct[:, _C_MASK : _C_MASK + 2].bitcast(_F32),
                axis=mybir.AxisListType.X,
                op=op.add,
            )

            # smooth-L1 = |e| + 0.5m^2 - m with m = min(|e|, 1). With
            # r = 0.5m that is |e| + 2(r^2 - r); the x2 folds into the
            # host-side sum. Unused/padded cells have e = 0 and
            # contribute nothing.
            e = ct[:, _C_E0 : _C_E0 + 12].bitcast(_F32)
            ae = pool.tile([M, 12], _F32)
            nc.vector.scalar_tensor_tensor(
                out=ae[:], in0=e, scalar=-1.0, in1=e,
                op0=op.mult, op1=op.max, accum_out=ps[:, 0:1],
            )
            rt = pool.tile([M, 12], _F32)
            nc.vector.tensor_scalar(
                out=rt[:], in0=ae[:], scalar1=1.0, scalar2=0.5,
                op0=op.min, op1=op.mult,
            )
            vt = pool.tile([M, 12], _F32)
            nc.vector.scalar_tensor_tensor(
                out=vt[:], in0=rt[:], scalar=-1.0, in1=rt[:],
                op0=op.add, op1=op.mult, accum_out=ps[:, 1:2],
            )

            nc.sync.dma_start(out=out_h.ap(), in_=ps[:])

    nc.compile()
    _BUILD_CACHE["nc"] = nc
    return nc


def _shard(inputs):
    """Build the 8 per-core input maps from the full inputs."""
    preds = [np.ascontiguousarray(inputs[f"pred_l{l}"], dtype=np.float32)
             for l in range(N_LVL)]
    coords = [np.ascontiguousarray(inputs[f"coord_l{l}"], dtype=np.int32)
              for l in range(N_LVL)]
    diffs = [np.ascontiguousarray(inputs[f"diff_l{l}"], dtype=np.float32)
             for l in range(N_LVL)]

    # per-batch chunk index/mask/diff, chunk = (anchor m, level l)
    lin_b = np.empty((B, M, N_LVL), dtype=np.int32)
    mask_b = np.empty((B, M, N_LVL), dtype=np.float32)
    ndiff_b = np.empty((B, M, N_LVL, N_CH), dtype=np.float32)
    for l in range(N_LVL):
        s = LEVEL_DIMS[l]
        c = coords[l]  # [B, M, 4]
        lin = (((c[:, :, 0] * s + c[:, :, 1]) * s + c[:, :, 2]) * (N_CH * s)
               + N_CH * c[:, :, 3] + _LVL_BASE[l])
        padded = c[:, :, 0] < 0
        lin_b[:, :, l] = np.where(padded, NP_TOT, lin)
        mask_b[:, :, l] = (~padded).astype(np.float32)
        # negated diff (gather adds g on top => e = g - d), zeroed on
        # padded rows so they contribute nothing
        ndiff_b[:, :, l, :] = -diffs[l] * mask_b[:, :, l : l + 1]

    # per-batch channel-last pred relayout: (6, A, S^3) -> (A, S^3, 6)
    pred_flat_b = []
    for b in range(B):
        blocks = []
        for l in range(N_LVL):
            s3 = LEVEL_DIMS[l] ** 3
            blk = preds[l][b].reshape(N_CH, A, s3)
            blocks.append(blk.transpose(1, 2, 0).reshape(-1))
        blocks.append(np.zeros(PRED_COLS, dtype=np.float32))
        pred_flat_b.append(
            np.concatenate(blocks).reshape(PRED_ROWS, PRED_COLS)
        )

    in_maps = []
    for core in range(N_CORES):
        b, mh = divmod(core, 2)
        # chunk k = 3*(m - 64*mh) + l for m in the core's anchor half
        ksl = slice(64 * mh, 64 * mh + 64)
        lin_k = lin_b[b, ksl].reshape(N_CHUNK)      # [192]
        mask_k = mask_b[b, ksl].reshape(N_CHUNK)
        nd_k = ndiff_b[b, ksl].reshape(N_CHUNK, N_CH)

        meta = np.zeros((M, META_COLS), dtype=np.int32)
        meta[:, _C_LIN0] = lin_k[:M]
        meta[:64, _C_LIN1] = lin_k[M:]
        meta[:, _C_MASK] = mask_k[:M].view(np.int32)
        meta[:64, _C_MASK + 1] = mask_k[M:].view(np.int32)
        meta[:, _C_E0 : _C_E0 + 6] = nd_k[:M].view(np.int32)
        meta[:64, _C_E1 : _C_E1 + 6] = nd_k[M:].view(np.int32)
        in_maps.append({"pred": pred_flat_b[b], "meta": meta})
    return in_maps


def run(inputs, trace=False, **kw):
    nc = _build()
    in_maps = _shard(inputs)
    res = bass_utils.run_bass_kernel_spmd(
        nc, in_maps, core_ids=list(range(N_CORES)), trace=trace, **kw
    )
    partials = np.stack([res.results[c]["out"] for c in range(N_CORES)])
    loss = np.float32(partials[:, :, 0].sum() + 2.0 * partials[:, :, 1].sum())
    weight = np.float32(partials[:, :, 2].sum())
    return (
        np.array([loss], dtype=np.float32),
        np.array([weight], dtype=np.float32),
    ), res


def kernel(**inputs):
    out, _ = run(inputs, trace=False)
    return out
